# revision 14
# baseline (speedup 1.0000x reference)
"""Trainium2 Bass kernel for nn_NeuromorphicSpikingNetwork (B=16, T=100).

Sharding (8 cores): Wa=W0 post-dim (512 cols/core), Wb=W1 post-dim (256
cols/core), both SBUF-resident f32 for the whole loop. Layer-0/1/2 LIF
state sharded the same way, b-major [16, width]. One AllGather per step
exchanges bf16 LR-scaled spikes + 2-way-split traces (b-major) and
bf16 transposed spikes (for the next step's contractions); layer 2 runs
one step behind layer 1 so a single gather suffices.

Numerics (validated against a numpy oracle): forward matmuls contract in
exact f32; STDP outer products use 2-way bf16 trace splits (error far
below the ~1e-5 spike-flip margin); the encoder uses host-precomputed
logit(u) so the device never evaluates sigmoid.
"""

import sys

sys.path.insert(0, "/opt/trn_rl_repo")
sys.path.insert(0, "/root/.axon_site/_ro/trn_rl_repo")

import numpy as np

import concourse.bass as bass
import concourse.bacc as bacc
import concourse.mybir as mybir
from concourse import tile
from concourse.bass_utils import run_bass_kernel_spmd

F32 = mybir.dt.float32
BF16 = mybir.dt.bfloat16
OP = mybir.AluOpType

B, D, T = 16, 2048, 100
N0, N1, N2 = 4096, 4096, 2048
THRESH, DECAY, REFRAC = 1.0, 0.9, 2
LR, TRACE_DECAY = 0.01, 0.95
NC_N = 8
J0, J1, J2 = N0 // NC_N, N1 // NC_N, N2 // NC_N   # 512, 512, 256
KC0, KC1, KD = N0 // 128, N1 // 128, D // 128      # 32, 32, 16

# AllGather payload per core, all bf16:
#   piece 0: LRs0 [16,512]   1: p0hi   2: p0lo   (layer-0, step t+1)
#   piece 3: LRs1 [16,512]   4: p1hi   5: p1lo   (layer-1, step t)
#   piece 6: s0T  [128,64]   7: s1T [128,64]  (transposed spikes, 0/1)
PIECE = B * J0          # elements per b-major piece (8192)
TP = 128 * 4 * B        # elements per transposed piece (8192)
CBE = 6 * PIECE + 2 * TP  # bf16 elements per core (81920? no: 6*8192+2*8192=65536)

_BUILT = {}
DEBUG = False


def build_kernel(t_steps):
    nc = bacc.Bacc("TRN2", target_bir_lowering=False, debug=False, num_devices=NC_N)

    xT_d = nc.dram_tensor("xT", [D, B], F32, kind="ExternalInput").ap()
    wenc_d = nc.dram_tensor("wenc", [D, J0], F32, kind="ExternalInput").ap()
    lu_d = nc.dram_tensor("lu", [B, t_steps, J0], F32, kind="ExternalInput").ap()
    wa_d = nc.dram_tensor("wa", [N0, J1], F32, kind="ExternalInput").ap()
    wb_d = nc.dram_tensor("wb", [N1, J2], F32, kind="ExternalInput").ap()
    eye_d = nc.dram_tensor("eye16", [B, B], F32, kind="ExternalInput").ap()
    counts_d = nc.dram_tensor("counts", [B, J2], F32, kind="ExternalOutput").ap()
    dbg_wa_d = nc.dram_tensor("dbg_wa", [N0, J1], F32, kind="ExternalOutput").ap() if DEBUG else None
    dbg_v1_d = nc.dram_tensor("dbg_v1", [B, J1], F32, kind="ExternalOutput").ap() if DEBUG else None
    dbg_q0_d = nc.dram_tensor("dbg_q0", [B, J1], F32, kind="ExternalOutput").ap() if DEBUG else None
    dbg_p0_d = nc.dram_tensor("dbg_p0", [B, J0], F32, kind="ExternalOutput").ap() if DEBUG else None
    dbg_c1_d = nc.dram_tensor("dbg_c1", [t_steps, B, J1], F32, kind="ExternalOutput").ap() if DEBUG else None
    dbg_s0T_d = nc.dram_tensor("dbg_s0T", [t_steps, 128, KC0 * B], F32, kind="ExternalOutput").ap() if DEBUG else None
    dbg_l0_d = nc.dram_tensor("dbg_l0", [t_steps, 4 * B, N0], BF16, kind="ExternalOutput").ap() if DEBUG else None
    dbg_mva_d = nc.dram_tensor("dbg_mva", [t_steps, 4 * B, J1], BF16, kind="ExternalOutput").ap() if DEBUG else None
    dbg_wat_d = nc.dram_tensor("dbg_wat", [t_steps, 128, 4 * J1], F32, kind="ExternalOutput").ap() if DEBUG else None

    with tile.TileContext(nc) as tc:
        with (
            tc.tile_pool(name="wts", bufs=1) as wts,
            tc.tile_pool(name="state", bufs=1) as st,
            tc.tile_pool(name="gath", bufs=1) as gp,
            tc.tile_pool(name="tmp", bufs=2) as tp_,
            tc.tile_pool(name="mmps", bufs=2, space="PSUM") as mmps,
            tc.tile_pool(name="trps", bufs=2, space="PSUM") as trps,
            tc.tile_pool(name="dwps", bufs=2, space="PSUM") as dwps,
            tc.tile_pool(name="dram", bufs=2, space="DRAM") as dr,
        ):
            # ---------------- resident data ----------------
            wa = wts.tile([128, KC0 * J1], F32)
            wb = wts.tile([128, KC1 * J2], F32)
            nc.sync.dma_start(out=wa[:].rearrange("p (k j) -> p k j", j=J1), in_=wa_d.rearrange("(k p) j -> p k j", p=128))
            nc.sync.dma_start(out=wb[:].rearrange("p (k j) -> p k j", j=J2), in_=wb_d.rearrange("(k p) j -> p k j", p=128))
            eye = wts.tile([B, B], F32)
            nc.sync.dma_start(out=eye[:], in_=eye_d[:, :])

            # ---------------- encoder (transient pool) ----------------
            z = st.tile([B, J0], F32)
            with tc.tile_pool(name="enc", bufs=1) as encp, tc.tile_pool(name="encw", bufs=3) as encw:
                xT = encp.tile([128, KD * B], F32)
                nc.sync.dma_start(out=xT[:].rearrange("p (k b) -> p k b", b=B), in_=xT_d.rearrange("(k p) b -> p k b", p=128))
                zp = mmps.tile([B, J0], F32, tag="mm")
                for k in range(KD):
                    wch = encw.tile([128, J0], F32, tag="wch")
                    nc.sync.dma_start(out=wch[:], in_=wenc_d[k * 128:(k + 1) * 128, :])
                    nc.tensor.matmul(zp[:], xT[:, k * B:(k + 1) * B], wch[:],
                                     start=(k == 0), stop=(k == KD - 1))
                nc.vector.tensor_copy(z[:], zp[:])

            # ---------------- state ----------------
            V0 = st.tile([B, J0], F32); r0 = st.tile([B, J0], F32); P0 = st.tile([B, J0], F32)
            V1 = st.tile([B, J1], F32); r1 = st.tile([B, J1], F32); P1 = st.tile([B, J1], F32)
            q0 = st.tile([B, J1], F32)
            V2 = st.tile([B, J2], F32); r2 = st.tile([B, J2], F32); q1 = st.tile([B, J2], F32)
            counts = st.tile([B, J2], F32)
            for tl in (V0, r0, P0, V1, r1, P1, q0, V2, r2, q1, counts):
                nc.vector.memset(tl[:], 0.0)

            # contribution staging (SBUF side of the gather)
            cbm = st.tile([B, 6 * J0], BF16)     # pieces 0..5
            ctr = st.tile([128, 8 * B], BF16)    # s0T | s1T
            # mov stacks for the outer products (cross-partition via HBM)
            movA = st.tile([4 * B, J1], BF16)    # [s1; s1; q0hi; q0lo]
            movB = st.tile([4 * B, J2], BF16)    # [s2; s2; q1hi; q1lo]
            mstage = st.tile([B, 3 * J1 + 3 * J2], BF16)
            mv_hbm = dr.tile([B, 3 * J1 + 3 * J2], BF16)

            def lif(V, r, cur_ap, width):
                spk = tp_.tile([B, width], F32, tag="spk%d" % width)
                m = tp_.tile([B, width], F32, tag="m%d" % width)
                nc.vector.scalar_tensor_tensor(V[:], V[:], float(DECAY), cur_ap, OP.mult, OP.add)
                nc.vector.tensor_scalar(m[:], V[:], float(THRESH), None, OP.is_gt)
                nc.vector.scalar_tensor_tensor(spk[:], r[:], 0.0, m[:], OP.is_le, OP.mult)
                # nspk = 1 - spk ; V *= nspk   (reuse m)
                nc.vector.tensor_scalar(m[:], spk[:], -1.0, 1.0, OP.mult, OP.add)
                nc.vector.tensor_tensor(V[:], V[:], m[:], OP.mult)
                # rm = max(r-1,0); r = max(2*spk, rm)
                nc.vector.tensor_scalar(r[:], r[:], 1.0, 0.0, OP.subtract, OP.max)
                nc.vector.scalar_tensor_tensor(r[:], spk[:], float(REFRAC), r[:], OP.mult, OP.max)
                return spk

            def split2(x, width, hi_ap, lo_ap, tag):
                r1_ = tp_.tile([B, width], F32, tag=tag)
                nc.vector.tensor_copy(hi_ap, x[:])
                nc.vector.scalar_tensor_tensor(r1_[:], hi_ap, -1.0, x[:], OP.mult, OP.add)
                nc.vector.tensor_copy(lo_ap, r1_[:])

            def transposes(spk, base_col):
                for i in range(4):
                    pt = trps.tile([128, B], F32, tag="tr")
                    nc.tensor.transpose(pt[:], spk[:, i * 128:(i + 1) * 128], eye[:])
                    nc.vector.tensor_copy(ctr[:, (base_col + i) * B:(base_col + i + 1) * B], pt[:])

            def lif0_contrib(t):
                sin = tp_.tile([B, J0], F32, tag="sin")
                nc.sync.dma_start(out=sin[:], in_=lu_d[:, t, :])
                nc.vector.tensor_tensor(sin[:], sin[:], z[:], OP.is_lt)
                spk0 = lif(V0, r0, sin[:], J0)
                nc.vector.tensor_scalar(cbm[:, 0:J0], spk0[:], -1.0, None, OP.mult)
                lrs = tp_.tile([B, J0], F32, tag="lrs0")
                nc.vector.tensor_scalar(lrs[:], spk0[:], float(LR), None, OP.mult)
                nc.vector.scalar_tensor_tensor(P0[:], P0[:], float(TRACE_DECAY), lrs[:], OP.mult, OP.add)
                split2(P0, J0, cbm[:, J0:2 * J0], cbm[:, 2 * J0:3 * J0], "sp0")
                transposes(spk0, 0)

            # ---------------- bootstrap: AG #0 carries L0 step 0 ----------------
            nc.vector.memset(cbm[:, 3 * J0:6 * J0], 0.0)
            nc.vector.memset(ctr[:, 4 * B:8 * B], 0.0)
            nc.vector.memset(mstage[:], 0.0)
            lif0_contrib(0)

            def launch_gather():
                cin = dr.tile([CBE], BF16, tag="cc_in")
                cout = dr.tile([NC_N * CBE], BF16, tag="cc_out")
                nc.sync.dma_start(out=cin[0:6 * PIECE].rearrange("(b n) -> b n", b=B), in_=cbm[:])
                nc.sync.dma_start(out=cin[6 * PIECE:].rearrange("(p f) -> p f", p=128), in_=ctr[:])
                nc.gpsimd.collective_compute(
                    "AllGather", OP.bypass,
                    replica_groups=[list(range(NC_N))],
                    ins=[cin.opt()], outs=[cout.opt()],
                )
                return cout

            def readback_L0(cout):
                """Return (L0stk [64, N0] bf16, s0T_f32 [128, KC0*B])."""
                L0 = gp.tile([4 * B, N0], BF16, tag="L0")
                co = cout[:].rearrange("(c e) -> c e", c=NC_N)
                bm = co[:, 0:6 * PIECE].rearrange("c (b i n) -> i b c n", b=B, i=6)
                for row, piece in ((0, 1), (1, 2), (2, 0), (3, 0)):
                    dst = L0[row * B:(row + 1) * B, :].rearrange("b (c n) -> b c n", c=NC_N)
                    nc.sync.dma_start(out=dst, in_=bm[piece])
                s0Tb = gp.tile([128, KC0 * B], BF16, tag="s0Tb")
                src = co[:, 6 * PIECE:].rearrange("c (p i f) -> i p c f", p=128, i=2)
                nc.sync.dma_start(out=s0Tb[:].rearrange("p (c f) -> p c f", c=NC_N), in_=src[0])
                s0T = gp.tile([128, KC0 * B], F32, tag="s0Tf")
                nc.vector.tensor_copy(s0T[:], s0Tb[:])
                return L0, s0T

            def readback_L1(cout):
                L1 = gp.tile([4 * B, N1], BF16, tag="L1")
                co = cout[:].rearrange("(c e) -> c e", c=NC_N)
                bm = co[:, 0:6 * PIECE].rearrange("c (b i n) -> i b c n", b=B, i=6)
                for row, piece in ((0, 4), (1, 5), (2, 3), (3, 3)):
                    dst = L1[row * B:(row + 1) * B, :].rearrange("b (c n) -> b c n", c=NC_N)
                    nc.sync.dma_start(out=dst, in_=bm[piece])
                s1Tb = gp.tile([128, KC1 * B], BF16, tag="s1Tb")
                src = co[:, 6 * PIECE:].rearrange("c (p i f) -> i p c f", p=128, i=2)
                nc.sync.dma_start(out=s1Tb[:].rearrange("p (c f) -> p c f", c=NC_N), in_=src[1])
                s1T = gp.tile([128, KC1 * B], F32, tag="s1Tf")
                nc.vector.tensor_copy(s1T[:], s1Tb[:])
                return L1, s1T

            cout_prev = launch_gather()

            GRP_A, GRP_B = 1, 2   # chunks per W-update DVE group

            for t in range(t_steps + 1):
                L0stk, s0T = (readback_L0(cout_prev)) if t < t_steps else (None, None)
                L1stk, s1T = (readback_L1(cout_prev)) if t >= 1 else (None, None)

                # ---- layer 2 for step t-1 ----
                if t >= 1:
                    c2 = mmps.tile([B, J2], F32, tag="mm")
                    for k in range(KC1):
                        nc.tensor.matmul(c2[:], s1T[:, k * B:(k + 1) * B],
                                         wb[:, k * J2:(k + 1) * J2],
                                         start=(k == 0), stop=(k == KC1 - 1))
                    spk2 = lif(V2, r2, c2[:], J2)
                    nc.vector.tensor_tensor(counts[:], counts[:], spk2[:], OP.add)
                    if t < t_steps:  # last step's dWb is never consumed
                        lrs2 = tp_.tile([B, J2], F32, tag="lrs2")
                        nc.vector.tensor_scalar(lrs2[:], spk2[:], float(LR), None, OP.mult)
                        nc.vector.scalar_tensor_tensor(q1[:], q1[:], float(TRACE_DECAY), lrs2[:], OP.mult, OP.add)
                        off = 3 * J1
                        nc.vector.tensor_copy(mstage[:, off:off + J2], spk2[:])
                        split2(q1, J2, mstage[:, off + J2:off + 2 * J2],
                               mstage[:, off + 2 * J2:off + 3 * J2], "sq1")

                # ---- layer 1 for step t ----
                if t < t_steps:
                    c1 = mmps.tile([B, J1], F32, tag="mm")
                    for k in range(KC0):
                        nc.tensor.matmul(c1[:], s0T[:, k * B:(k + 1) * B],
                                         wa[:, k * J1:(k + 1) * J1],
                                         start=(k == 0), stop=(k == KC0 - 1))
                    if DEBUG:
                        dcp = tp_.tile([B, J1], F32, tag="dbgc1")
                        nc.vector.tensor_copy(dcp[:], c1[:])
                        nc.sync.dma_start(out=dbg_c1_d[t], in_=dcp[:])
                        nc.sync.dma_start(out=dbg_s0T_d[t], in_=s0T[:])
                    spk1 = lif(V1, r1, c1[:], J1)
                    nc.vector.tensor_scalar(cbm[:, 3 * J0:4 * J0], spk1[:], -1.0, None, OP.mult)
                    lrs1 = tp_.tile([B, J1], F32, tag="lrs1")
                    nc.vector.tensor_scalar(lrs1[:], spk1[:], float(LR), None, OP.mult)
                    nc.vector.scalar_tensor_tensor(P1[:], P1[:], float(TRACE_DECAY), lrs1[:], OP.mult, OP.add)
                    split2(P1, J1, cbm[:, 4 * J0:5 * J0], cbm[:, 5 * J0:6 * J0], "sp1")
                    nc.vector.scalar_tensor_tensor(q0[:], q0[:], float(TRACE_DECAY), lrs1[:], OP.mult, OP.add)
                    nc.vector.tensor_copy(mstage[:, 0:J1], spk1[:])
                    split2(q0, J1, mstage[:, J1:2 * J1], mstage[:, 2 * J1:3 * J1], "sq0")
                    transposes(spk1, 4)

                    # ---- layer 0 for step t+1 ----
                    if t + 1 < t_steps:
                        lif0_contrib(t + 1)

                    cout_prev = launch_gather()

                    # ---- mov stacks via private HBM round-trip ----
                    mv = dr.tile([B, 3 * J1 + 3 * J2], BF16, tag="mv")
                    nc.sync.dma_start(out=mv[:, :], in_=mstage[:])
                    nc.sync.dma_start(out=movA[0:B, :], in_=mv[:, 0:J1])
                    nc.sync.dma_start(out=movA[B:2 * B, :], in_=mv[:, 0:J1])
                    nc.sync.dma_start(out=movA[2 * B:3 * B, :], in_=mv[:, J1:2 * J1])
                    nc.sync.dma_start(out=movA[3 * B:4 * B, :], in_=mv[:, 2 * J1:3 * J1])
                    if t >= 1:
                        off = 3 * J1
                        nc.sync.dma_start(out=movB[0:B, :], in_=mv[:, off:off + J2])
                        nc.sync.dma_start(out=movB[B:2 * B, :], in_=mv[:, off:off + J2])
                        nc.sync.dma_start(out=movB[2 * B:3 * B, :], in_=mv[:, off + J2:off + 2 * J2])
                        nc.sync.dma_start(out=movB[3 * B:4 * B, :], in_=mv[:, off + 2 * J2:off + 3 * J2])

                    if DEBUG:
                        nc.sync.dma_start(out=dbg_l0_d[t], in_=L0stk[:])
                        nc.sync.dma_start(out=dbg_mva_d[t], in_=movA[:])
                    # ---- dWa + Wa update ----
                    for g in range(KC0 // GRP_A):
                        pd = dwps.tile([128, GRP_A * J1], F32, tag="pda")
                        for i in range(GRP_A):
                            k = g * GRP_A + i
                            nc.tensor.matmul(pd[:, i * J1:(i + 1) * J1],
                                             L0stk[:, k * 128:(k + 1) * 128],
                                             movA[:, :], start=True, stop=True)
                        ws = wa[:, g * GRP_A * J1:(g + 1) * GRP_A * J1]
                        nc.vector.scalar_tensor_tensor(ws, pd[:], 1.0, ws, OP.mult, OP.add)
                        nc.vector.tensor_scalar(ws, ws, 1.0, -1.0, OP.min, OP.max)

                    if DEBUG:
                        nc.sync.dma_start(out=dbg_wat_d[t], in_=wa[:, 0:4 * J1])
                    # ---- dWb + Wb update (step t-1) ----
                    if t >= 1:
                        for g in range(KC1 // GRP_B):
                            pd = dwps.tile([128, GRP_B * J2], F32, tag="pdb")
                            for i in range(GRP_B):
                                k = g * GRP_B + i
                                nc.tensor.matmul(pd[:, i * J2:(i + 1) * J2],
                                                 L1stk[:, k * 128:(k + 1) * 128],
                                                 movB[:, :], start=True, stop=True)
                            ws = wb[:, g * GRP_B * J2:(g + 1) * GRP_B * J2]
                            nc.vector.scalar_tensor_tensor(ws, pd[:], 1.0, ws, OP.mult, OP.add)
                            nc.vector.tensor_scalar(ws, ws, 1.0, -1.0, OP.min, OP.max)

            nc.sync.dma_start(out=counts_d[:, :], in_=counts[:])
            if DEBUG:
                nc.sync.dma_start(out=dbg_wa_d.rearrange("(k p) j -> p k j", p=128),
                                  in_=wa[:].rearrange("p (k j) -> p k j", j=J1))
                nc.sync.dma_start(out=dbg_v1_d[:, :], in_=V1[:])
                nc.sync.dma_start(out=dbg_q0_d[:, :], in_=q0[:])
                nc.sync.dma_start(out=dbg_p0_d[:, :], in_=P0[:])

    nc.compile()
    return nc


def _logit(u):
    u64 = u.astype(np.float64)
    return (np.log(u64) - np.log1p(-u64)).astype(np.float32)


def kernel(x, u, W_enc, b_enc, W0, W1, W_dec, b_dec, t_steps=T):
    x = np.asarray(x); u = np.asarray(u)
    W_enc = np.asarray(W_enc); W0 = np.asarray(W0); W1 = np.asarray(W1)
    W_dec = np.asarray(W_dec)
    b_enc = np.asarray(b_enc); b_dec = np.asarray(b_dec)

    if t_steps not in _BUILT:
        _BUILT[t_steps] = build_kernel(t_steps)
    nc = _BUILT[t_steps]

    lu = _logit(u[:, :t_steps, :])
    xT = np.ascontiguousarray(x.T)
    eye = np.eye(B, dtype=np.float32)
    in_maps = []
    for c in range(NC_N):
        in_maps.append({
            "xT": xT,
            "wenc": np.ascontiguousarray(W_enc[:, c * J0:(c + 1) * J0]),
            "lu": np.ascontiguousarray(lu[:, :, c * J0:(c + 1) * J0]),
            "wa": np.ascontiguousarray(W0[:, c * J1:(c + 1) * J1]),
            "wb": np.ascontiguousarray(W1[:, c * J2:(c + 1) * J2]),
            "eye16": eye,
        })
    # b_enc is all zeros in this problem; fold anyway via wenc shift if nonzero
    if np.any(b_enc != 0):
        # add b_enc by shifting lu: u < sigmoid(z + b) <=> logit(u) - b < z
        for c in range(NC_N):
            in_maps[c]["lu"] = in_maps[c]["lu"] - b_enc[c * J0:(c + 1) * J0][None, None, :]

    res = run_bass_kernel_spmd(nc, in_maps, list(range(NC_N)))
    cnt = np.concatenate([res.results[c]["counts"] for c in range(NC_N)], axis=1)
    global _last_counts, _last_res
    _last_counts = cnt
    _last_res = res
    return (cnt @ W_dec + b_dec).astype(np.float32)


if __name__ == "__main__":
    rng = np.random.RandomState(0)
    pass
